# revision 32
# baseline (speedup 1.0000x reference)
"""Trainium2 Bass kernel for nn_Attention_50354196578449 (sparse_attention).

Reference computation (per batch b of B=64, N=512, MD=QD=AD=1024):
    tq      = query @ Ws                                   # (B, AD)
    h       = tanh(memory_values @ Wh + tq[:, None, :])    # (B, N, AD)
    logits  = squeeze(h @ v)                               # (B, N)
    weights = masked softmax(logits)                       # (B, N)
    context = einsum("bn,bnd->bd", weights, memory_values) # (B, MD)

Strategy: data-parallel over batch across 8 NeuronCores (8 batches/core).

Two levers on top of the fused fp16 pipeline:
  - sparsity: rows with mask==0 get -1e30 logits, so their h/logit work is
    dead. Host-side we gather each batch's unmasked rows to the front
    (padding with masked rows, suppressed via a host-built additive
    vector) and only compute NPAD ~ 288 of the 512 rows on device.
  - fp8 DoubleRow: the A-phase (mv @ Wh) and tq (q @ Ws) run as
    float8e4 matmuls in DoubleRow perf mode (2 K-tiles per instruction,
    0.5 cyc/row => 2x PE throughput). Inputs are pre-scaled by powers of
    two (mv,q x32; Wh,Ws x4096) so the fp8e4 normal range is used; the
    2^-17 product scale is folded into the tanh activation's scale and
    the tq PSUM->SBUF copy. Everything downstream of tanh (logits via
    fp16 h @ v, softmax in fp32, context via fp16 mv) stays >= fp16,
    which the softmax/context accuracy actually needs.

Per core, fully on-chip, software-pipelined across engines:
  - host pre-transposes everything (no on-chip transposes at all):
    mvT8 [p(md_lo), g, i, n] fp8 for the A-phase moving operand,
    mvT16 [p(md_lo), mdc, n] fp16 for the DVE context contraction,
    Wh8/Ws8 [p, adc, g, i, a_lo] fp8 stationary tiles, qT8, v16.
  - A-phase: per (batch, adc) one PSUM tile [128, NPAD] accumulates 4
    DoubleRow matmuls (K=256 each); ACT applies tanh with scale=2^-17
    and per-partition bias tq^T[:, b] -> hT fp16.
  - logits accumulate in PSUM via v-chunk (M=1) fp16 matmuls over ad,
    lagging the A-phase by three chunks so the ~600ns tanh and its
    semaphore hops never stall the in-order PE queue.
  - batch b's softmax (DVE/ACT/gpsimd small ops), context contraction
    (gpsimd partition_broadcast + fused DVE affine_mul_reduce against
    the resident mvT16 tiles) and context^T output (PE transpose) are
    emitted inside batches b+1/b+2's A-loops, so the in-order ACT/DVE
    queues never make the PE wait at batch boundaries.
  - DMA rails: ACT's queue carries only the four tiny prologue loads
    (DMA trigger instructions cost ~0.7us of engine time and would
    stall the tanh stream); all bulk traffic rides the SP HWDGE rail
    (weights blocks interleaved in first-use order, then even batches)
    and the gpsimd SWDGE rail (odd batches).
  - tq runs once as fp8 DoubleRow matmuls interleaved into batch 0's
    A-phase, while the Ws8 blocks stream in.
  - a short PE warmup (pinned first via a PSUM WAW dep + explicit
    ordering edges) keeps the PE clock up while the prologue DMAs land.
"""

import sys

sys.path.insert(0, "/opt/trn_rl_repo")

from contextlib import ExitStack

import numpy as np

N_CORES = 8
B = 64
B_LOC = B // N_CORES  # 8 batches per core
N = 512
MD = 1024
QD = 1024
AD = 1024
P = 128
NG = 4         # DoubleRow K-groups over md/qd (4 x 256 = 1024)
NAD = AD // P  # 8 ad chunks
NMD = MD // P  # 8 md chunks
S_MV = 32.0
S_WH = 4096.0
S_Q = 32.0
S_WS = 4096.0
INV_A = 1.0 / (S_MV * S_WH)   # 2^-17, folded into tanh scale
INV_TQ = 1.0 / (S_Q * S_WS)   # 2^-17, folded into tq copy scale
WARMUP_MMS = 28
LAG = 3        # logits matmul lag (in A-groups) behind the tanh

_CACHE = {}


def _build_nc(npad):
    import concourse.bass as bass  # noqa: F401
    import concourse.tile as tile
    from concourse import bacc, mybir
    from concourse.masks import make_identity

    F32 = mybir.dt.float32
    F16 = mybir.dt.float16
    F8 = mybir.dt.float8e4
    AF = mybir.ActivationFunctionType
    OP = mybir.AluOpType
    AX = mybir.AxisListType
    DR = mybir.MatmulPerfMode.DoubleRow

    nc = bacc.Bacc("TRN2", target_bir_lowering=False)

    mvT8_d = nc.dram_tensor("mvT8", (B_LOC, P, NG, 2, npad), F8,
                            kind="ExternalInput")
    mvT16_d = nc.dram_tensor("mvT16", (B_LOC, P, NMD, npad), F16,
                             kind="ExternalInput")
    Wh8_d = nc.dram_tensor("Wh8", (P, NAD, NG, 2, P), F8,
                           kind="ExternalInput")
    Ws8_d = nc.dram_tensor("Ws8", (P, NAD, NG, 2, P), F8,
                           kind="ExternalInput")
    qT8_d = nc.dram_tensor("qT8", (P, NG, 2, B_LOC), F8,
                           kind="ExternalInput")
    v_d = nc.dram_tensor("v16", (P, NAD), F16, kind="ExternalInput")
    sup_d = nc.dram_tensor("sup", (1, B_LOC, npad), F32,
                           kind="ExternalInput")
    mx_d = nc.dram_tensor("mx", (1, B_LOC), F32, kind="ExternalInput")
    mvN_d = nc.dram_tensor("mvN", (2, npad, MD), F16, kind="ExternalInput")
    ctx_d = nc.dram_tensor("context", (B_LOC, MD), F32,
                           kind="ExternalOutput")
    scratch_d = nc.dram_tensor("scratch", (1, 2), F32,
                               kind="ExternalOutput")
    nch = -(-npad // P)  # row chunks of the natural-layout mv tail tiles

    with tile.TileContext(nc) as tc, ExitStack() as ctx:
        const = ctx.enter_context(tc.tile_pool(name="const", bufs=1))
        mvT8_pool = ctx.enter_context(tc.tile_pool(name="mvT8", bufs=3))
        mvT16_pool = ctx.enter_context(tc.tile_pool(name="mvT16", bufs=3))
        hT_pool = ctx.enter_context(tc.tile_pool(name="hT", bufs=6))
        small = ctx.enter_context(tc.tile_pool(name="small", bufs=2))
        dpool = ctx.enter_context(tc.tile_pool(name="dpool", bufs=3))
        out_pool = ctx.enter_context(tc.tile_pool(name="outp", bufs=2))
        natN_pool = ctx.enter_context(tc.tile_pool(name="natN", bufs=2))
        psum_h = ctx.enter_context(
            tc.tile_pool(name="psum_h", bufs=4, space="PSUM"))
        psum_log = ctx.enter_context(
            tc.tile_pool(name="psum_log", bufs=2, space="PSUM"))
        psum_tr = ctx.enter_context(
            tc.tile_pool(name="psum_tr", bufs=1, space="PSUM"))
        psum_keep = ctx.enter_context(
            tc.tile_pool(name="psum_keep", bufs=1, space="PSUM"))

        # ---- identities + PE warmup (keeps the PE clock up while the
        # ---- prologue DMAs stream in) -------------------------------------
        ident_f = const.tile([P, P], F32)
        make_identity(nc, ident_f[:])
        ident_h = const.tile([P, P], F16)
        make_identity(nc, ident_h[:])

        import bass_rust as _br

        ps_h0 = psum_h.tile([P, npad], F32, name="ps_h", tag="ps_h")
        last_warm = None
        for _ in range(WARMUP_MMS):
            last_warm = nc.tensor.matmul(ps_h0[:, 0:P], ident_h[:],
                                         ident_h[:], start=True, stop=True,
                                         skip_group_check=True)

        def after_warmup(bi):
            _br.add_dep_helper(bi.ins, last_warm.ins, sync=False,
                               reason="keep warmup at the head of the PE stream")
            return bi

        # ---- tiny loads: the ONLY traffic on the ACT rail ------------------
        qT8_sb = const.tile([P, NG, 2, B_LOC], F8)
        nc.scalar.dma_start(qT8_sb[:], qT8_d[:])
        v_sb = const.tile([P, NAD], F16)
        nc.scalar.dma_start(v_sb[:], v_d[:])
        sup_sb = const.tile([1, B_LOC, npad], F32)
        nc.scalar.dma_start(sup_sb[:], sup_d[:])
        mx_sb = const.tile([1, B_LOC], F32)
        nc.scalar.dma_start(mx_sb[:], mx_d[:])

        # ---- SP-rail prologue: weight blocks interleaved in first-use
        # ---- order, then batch-0 mv tiles ---------------------------------
        Ws8_sb = const.tile([P, NAD, NG, 2, P], F8)
        Wh8_sb = const.tile([P, NAD, NG, 2, P], F8)
        # Ws8 rides the ACT HWDGE rail: its trigger instructions execute
        # during the preamble while the ACT queue is otherwise idle, so
        # they never block the tanh stream
        nc.scalar.dma_start(Ws8_sb[:, 0:1], Ws8_d[:, 0:1])
        nc.scalar.dma_start(Ws8_sb[:, 1:2], Ws8_d[:, 1:2])
        nc.scalar.dma_start(Ws8_sb[:, 2:4], Ws8_d[:, 2:4])
        nc.scalar.dma_start(Ws8_sb[:, 4:8], Ws8_d[:, 4:8])
        nc.sync.dma_start(Wh8_sb[:, 0:2], Wh8_d[:, 0:2])

        mvT8s = [None] * B_LOC
        mvT16s = [None] * B_LOC

        def emit_loads(b):
            """mvT8 (A-phase) + mvT16 (context) loads for batch b.

            Batch 0 rides the SP HWDGE rail in g-chunks (its first chunk
            gates the first real matmul); odd batches ride the SWDGE
            rail, later even batches the SP rail, always with a full
            batch of slack."""
            mvT8 = mvT8_pool.tile([P, NG, 2, npad], F8, tag="mvT8")
            mvT16 = mvT16_pool.tile([P, NMD, npad], F16, tag="mvT16")
            if b == 0:
                for g in range(NG):
                    nc.sync.dma_start(mvT8[:, g], mvT8_d[b, :, g])
            elif b % 2 == 1:
                nc.gpsimd.dma_start(mvT8[:], mvT8_d[b])
                nc.gpsimd.dma_start(mvT16[:], mvT16_d[b])
            else:
                nc.sync.dma_start(mvT8[:], mvT8_d[b])
                nc.sync.dma_start(mvT16[:], mvT16_d[b])
            mvT8s[b] = mvT8
            mvT16s[b] = mvT16

        emit_loads(0)
        nc.sync.dma_start(Wh8_sb[:, 2:4], Wh8_d[:, 2:4])
        nc.sync.dma_start(Wh8_sb[:, 4:8], Wh8_d[:, 4:8])
        nc.sync.dma_start(mvT16s[0][:], mvT16_d[0])

        ones_h = const.tile([1, P], F16)
        nc.gpsimd.memset(ones_h[:], 1.0)

        # HAM keep-alive: the PE downclocks to half speed (k=4/8) within
        # ~4us of going idle, which would stretch every op in a stalled
        # region ~2x. Dummy matmuls into a scratch PSUM bank burn the
        # idle slots of DMA-gated (b0) and drain (tail) phases so the
        # clock stays up; they cost nothing when the PE queue is full.
        # They ACCUMULATE into one never-closed PSUM group that a final
        # read drains to a scratch output, so dead-code elimination
        # cannot drop them.
        ps_keep = psum_keep.tile([P, P], F32, name="ps_keep")
        ka_open = [False]

        def keep_alive(n, close=False):
            for k in range(n):
                nc.tensor.matmul(ps_keep[:], ident_h[:], ident_h[:],
                                 start=not ka_open[0],
                                 stop=close and k == n - 1,
                                 skip_group_check=True)
                ka_open[0] = True

        # ---- tq^T columns, fp8 DoubleRow, interleaved into batch 0 --------
        tqT_sb = const.tile([P, NAD, B_LOC], F32)

        def emit_tq(adc):
            ps_tq = psum_tr.tile([P, B_LOC], F32, tag="tr", name="ps_tq")
            for g in range(NG):
                after_warmup(nc.tensor.matmul(
                    ps_tq[:], Ws8_sb[:, adc, g], qT8_sb[:, g],
                    start=(g == 0), stop=(g == NG - 1), perf_mode=DR,
                    skip_group_check=True))
            nc.scalar.activation(tqT_sb[:, adc, :], ps_tq[:], AF.Copy,
                                 scale=INV_TQ)

        wbs = [None] * B_LOC
        ctxs = [None] * B_LOC
        ps_logs = [None] * B_LOC
        hts = {}

        def emit_logit(gi):
            bb, k = divmod(gi, NAD)
            nc.tensor.matmul(ps_logs[bb][:], v_sb[:, k:k + 1], hts.pop(gi),
                             start=(k == 0), stop=(k == NAD - 1),
                             skip_group_check=True)

        def emit_softmax(b):
            """masked softmax on partition 0 (sup/mx host-precomputed).

            No max-subtraction: |logits| <= ||v||_1 ~ 18 so fp32 exp
            cannot overflow, and suppressed (-1e30) entries underflow to
            exactly 0. The 1e-30 epsilon keeps 1/sum finite in the
            all-masked edge case (weights are then zeroed via mx)."""
            ml = small.tile([1, npad], F32, tag="ml")
            nc.vector.scalar_tensor_tensor(
                ml[:], in0=sup_sb[0:1, b, :], scalar=mx_sb[0:1, b:b + 1],
                in1=ps_logs[b][:], op0=OP.mult, op1=OP.add)
            et = small.tile([1, npad], F32, tag="et")
            zs = small.tile([1, 1], F32, tag="zs")
            nc.scalar.activation(et[:], ml[:], AF.Exp, accum_out=zs[:])
            zse = small.tile([1, 1], F32, tag="zse")
            nc.vector.tensor_scalar(zse[:], zs[:], 1.0, 1e-30,
                                    op0=OP.mult, op1=OP.add)
            rz = small.tile([1, 1], F32, tag="rz")
            nc.vector.reciprocal(rz[:], zse[:])
            wb = small.tile([1, npad], F16, tag="wb")
            nc.vector.tensor_scalar(wb[:], et[:], rz[:],
                                    mx_sb[0:1, b:b + 1],
                                    op0=OP.mult, op1=OP.mult)
            wbs[b] = wb

        def emit_D_compute(b):
            """context^T[md, b]: broadcast w(b) across partitions via a
            rank-1 PE matmul (ones x w) + ACT copy (both engines have
            slack), then one fused DVE multiply+reduce per md chunk
            against the resident mvT16."""
            ps_wbc = psum_tr.tile([P, npad], F32, tag="tr", name="ps_wbc")
            nc.tensor.matmul(ps_wbc[:], ones_h[:], wbs[b][:],
                             start=True, stop=True, skip_group_check=True)
            wbc = dpool.tile([P, npad], F16, tag="wbc")
            nc.scalar.copy(wbc[:], ps_wbc[:])
            ctx_b = dpool.tile([P, NMD], F32, tag="ctxb")
            for mdc in range(NMD):
                scr = dpool.tile([P, npad], F16, tag="dscr")
                nc.vector.affine_mul_reduce(scr[:], ctx_b[:, mdc:mdc + 1],
                                            mvT16s[b][:, mdc], wbc[:],
                                            1.0, 0.0)
            ctxs[b] = ctx_b

        def emit_D_out(b):
            """ctx^T [128(md_lo), 8(mdc)] -> [8, 128] -> DRAM."""
            ps_c = psum_tr.tile([B_LOC, P], F32, tag="tr", name="ps_c")
            nc.tensor.transpose(ps_c[:], ctxs[b][:], ident_f[:P, :P])
            out_sb = out_pool.tile([NMD, P], F32)
            nc.vector.tensor_copy(out_sb[:], ps_c[:])
            nc.sync.dma_start(
                ctx_d[b:b + 1, :].rearrange("x (c p) -> (x c) p", p=P),
                out_sb[:])

        # ---- natural-layout mv for the last two batches: their context
        # ---- runs on the (tail-idle) PE instead of the DVE ----------------
        natNs = [None, None]

        def emit_natN(j):
            natN = natN_pool.tile([P, nch, MD], F16, tag="natN")
            for t in range(nch):
                r0, r1 = t * P, min((t + 1) * P, npad)
                nc.sync.dma_start(natN[0:r1 - r0, t, :],
                                  mvN_d[j, r0:r1, :])
            natNs[j] = natN

        def emit_D_pe(b, j):
            """context[b] = w @ mv_nat via M=1 PE matmuls (tail path)."""
            ps_wT = psum_tr.tile([P, nch, 2], F16, tag="tr", name="ps_wT")
            for t in range(nch):
                r0, r1 = t * P, min((t + 1) * P, npad)
                nc.tensor.matmul(ps_wT[0:r1 - r0, t, 0:1],
                                 wbs[b][0:1, r0:r1], ident_h[0:1, 0:1],
                                 is_transpose=True, skip_group_check=True)
            wT = dpool.tile([P, nch, 2], F16, tag="wT")
            nc.vector.tensor_copy(wT[:, :, 0:1], ps_wT[:, :, 0:1])
            out_nat = out_pool.tile([1, MD], F32, name="out_nat",
                                    tag="onat")
            for half in range(2):
                ps_c2 = psum_tr.tile([1, MD // 2], F32, tag="tr",
                                     name="ps_c2")
                for t in range(nch):
                    r0, r1 = t * P, min((t + 1) * P, npad)
                    nc.tensor.matmul(
                        ps_c2[:], wT[0:r1 - r0, t, 0:1],
                        natNs[j][0:r1 - r0, t,
                                 half * (MD // 2):(half + 1) * (MD // 2)],
                        start=(t == 0), stop=(t == nch - 1),
                        skip_group_check=True)
                nc.vector.tensor_copy(
                    out_nat[0:1, half * (MD // 2):(half + 1) * (MD // 2)],
                    ps_c2[:])
            nc.sync.dma_start(ctx_d[b:b + 1, :], out_nat[:])

        for b in range(B_LOC):
            if b + 1 < B_LOC:
                emit_loads(b + 1)

            mvT8 = mvT8s[b]
            ps_logs[b] = psum_log.tile([1, npad], F32, tag="log",
                                       name="ps_log")
            for adc in range(NAD):
                gi = b * NAD + adc
                if b == 0:
                    emit_tq(adc)
                if gi == 0:
                    ps_h = ps_h0
                else:
                    ps_h = psum_h.tile([P, npad], F32, name="ps_h",
                                       tag="ps_h")
                for g in range(NG):
                    mm = nc.tensor.matmul(
                        ps_h[:], Wh8_sb[:, adc, g], mvT8[:, g],
                        start=(g == 0), stop=(g == NG - 1), perf_mode=DR,
                        skip_group_check=True)
                    if b == 0:
                        after_warmup(mm)
                hT = hT_pool.tile([P, npad], F16)
                nc.scalar.activation(hT[:], ps_h[:], AF.Tanh,
                                     bias=tqT_sb[:, adc, b:b + 1],
                                     scale=INV_A)
                hts[gi] = hT
                if b == 0:
                    keep_alive(4)
                if gi >= LAG:
                    emit_logit(gi - LAG)
                if b > 0:
                    if adc == 3:
                        emit_softmax(b - 1)
                    elif adc == 5 and b - 1 < B_LOC - 2:
                        emit_D_compute(b - 1)
                    elif adc == 1 and b > 1 and b - 2 < B_LOC - 2:
                        emit_D_out(b - 2)
                    elif adc == 6 and b in (B_LOC - 3, B_LOC - 2):
                        emit_natN(b - (B_LOC - 3))

        for gi in range(B_LOC * NAD - LAG, B_LOC * NAD):
            emit_logit(gi)
        # tail: batch 6's context runs on the now-idle PE right away
        # (wb(6) has been ready since adc 3); softmax(7) drains on
        # DVE/ACT in parallel; keep-alive bursts pad the PE queue so the
        # clock stays up until the last output is computed
        emit_D_pe(B_LOC - 2, 0)
        emit_softmax(B_LOC - 1)
        keep_alive(16)
        emit_D_pe(B_LOC - 1, 1)
        keep_alive(8, close=True)
        ka_sb = out_pool.tile([1, 2], F32, name="ka_sb")
        nc.vector.tensor_copy(ka_sb[:], ps_keep[0:1, 0:2])
        nc.sync.dma_start(scratch_d[:], ka_sb[:])

    nc.compile()
    return nc


def _get_nc(npad):
    key = ("nc", npad)
    if key not in _CACHE:
        _CACHE[key] = _build_nc(npad)
    return _CACHE[key]


def _quant8(x, scale):
    import ml_dtypes

    return np.clip(x * scale, -240.0, 240.0).astype(ml_dtypes.float8_e4m3)


def _prep(memory_values, mask, query, Wh, Ws, v):
    """Host-side dtype/layout prep. Returns (nc, per-core input maps)."""
    memory_values = np.ascontiguousarray(memory_values, dtype=np.float32)
    mask = np.ascontiguousarray(mask)
    query = np.ascontiguousarray(query, dtype=np.float32)
    Wh = np.ascontiguousarray(Wh, dtype=np.float32)
    Ws = np.ascontiguousarray(Ws, dtype=np.float32)
    v = np.ascontiguousarray(v, dtype=np.float32)

    # ---- host prep: gather unmasked rows to the front ----------------------
    unmasked = mask != 0
    counts = unmasked.sum(axis=1).astype(np.int64)
    npad = int(min(N, max(192, -(-int(counts.max()) // 32) * 32)))
    # unmasked rows first (stable order); pads are real masked rows whose
    # logits the suppression vector kills, so no zero-fill is needed.
    order = np.argsort(~unmasked, axis=1, kind="stable")[:, :npad]
    mv_g = np.take_along_axis(memory_values, order[:, :, None], axis=1)

    mv8 = _quant8(mv_g, S_MV)                       # [B, npad, MD]
    mvT8 = mv8.reshape(B, npad, NG, 2, P).transpose(0, 4, 2, 3, 1)
    mvT8 = np.ascontiguousarray(mvT8)               # [B, P, NG, 2, npad]
    mv16 = mv_g.astype(np.float16)
    mvT16 = mv16.reshape(B, npad, NMD, P).transpose(0, 3, 2, 1)
    mvT16 = np.ascontiguousarray(mvT16)             # [B, P, NMD, npad]

    # weights: [k, a] -> [p, adc, g, i, a_lo] with k = g*256 + i*128 + p
    def wprep(W, scale):
        w8 = _quant8(W, scale).reshape(NG, 2, P, NAD, P)
        return np.ascontiguousarray(w8.transpose(2, 3, 0, 1, 4))

    Wh8 = wprep(Wh, S_WH)
    Ws8 = wprep(Ws, S_WS)
    qT8 = np.ascontiguousarray(
        _quant8(query, S_Q).reshape(B, NG, 2, P).transpose(3, 1, 2, 0))
    v16 = np.ascontiguousarray(v[:, 0].reshape(NAD, P).T.astype(np.float16))

    sup = np.where(np.arange(npad)[None, :] < counts[:, None],
                   np.float32(0.0), np.float32(-1e30))
    mx = (counts > 0).astype(np.float32)

    nc = _get_nc(npad)
    in_maps = []
    for c in range(N_CORES):
        s = slice(c * B_LOC, (c + 1) * B_LOC)
        in_maps.append({
            "mvT8": mvT8[s],
            "mvT16": mvT16[s],
            "Wh8": Wh8,
            "Ws8": Ws8,
            "qT8": qT8[:, :, :, s],
            "v16": v16,
            "sup": sup[None, s],
            "mx": mx[None, s],
            "mvN": mv16[s][B_LOC - 2:],
        })
    return nc, in_maps


def kernel(memory_values, mask, query, Wh, Ws, v):
    from concourse.bass_utils import run_bass_kernel_spmd

    nc, in_maps = _prep(memory_values, mask, query, Wh, Ws, v)
    res = run_bass_kernel_spmd(nc, in_maps, core_ids=list(range(N_CORES)))
    out = np.concatenate([res.results[c]["context"] for c in range(N_CORES)],
                         axis=0)
    return out.astype(np.float32)


# revision 36
# speedup vs baseline: 1.0446x; 1.0446x over previous
"""Trainium2 Bass kernel for nn_Attention_50354196578449 (sparse_attention).

Reference computation (per batch b of B=64, N=512, MD=QD=AD=1024):
    tq      = query @ Ws                                   # (B, AD)
    h       = tanh(memory_values @ Wh + tq[:, None, :])    # (B, N, AD)
    logits  = squeeze(h @ v)                               # (B, N)
    weights = masked softmax(logits)                       # (B, N)
    context = einsum("bn,bnd->bd", weights, memory_values) # (B, MD)

Strategy: data-parallel over batch across 8 NeuronCores (8 batches/core).

Two levers on top of the fused fp16 pipeline:
  - sparsity: rows with mask==0 get -1e30 logits, so their h/logit work is
    dead. Host-side we gather each batch's unmasked rows to the front
    (padding with masked rows, suppressed via a host-built additive
    vector) and only compute NPAD ~ 288 of the 512 rows on device.
  - fp8 DoubleRow: the A-phase (mv @ Wh) and tq (q @ Ws) run as
    float8e4 matmuls in DoubleRow perf mode (2 K-tiles per instruction,
    0.5 cyc/row => 2x PE throughput). Inputs are pre-scaled by powers of
    two (mv,q x32; Wh,Ws x4096) so the fp8e4 normal range is used; the
    2^-17 product scale is folded into the tanh activation's scale and
    the tq PSUM->SBUF copy. Everything downstream of tanh (logits via
    fp16 h @ v, softmax in fp32, context via fp16 mv) stays >= fp16,
    which the softmax/context accuracy actually needs.

Per core, fully on-chip, software-pipelined across engines:
  - host pre-transposes everything (no on-chip transposes at all):
    mvT8 [p(md_lo), g, i, n] fp8 for the A-phase moving operand,
    mvT16 [p(md_lo), mdc, n] fp16 for the DVE context contraction,
    Wh8/Ws8 [p, adc, g, i, a_lo] fp8 stationary tiles, qT8, v16.
  - A-phase: per (batch, adc) one PSUM tile [128, NPAD] accumulates 4
    DoubleRow matmuls (K=256 each); ACT applies tanh with scale=2^-17
    and per-partition bias tq^T[:, b] -> hT fp16.
  - logits accumulate in PSUM via v-chunk (M=1) fp16 matmuls over ad,
    lagging the A-phase by three chunks so the ~600ns tanh and its
    semaphore hops never stall the in-order PE queue.
  - batch b's softmax (DVE/ACT/gpsimd small ops), context contraction
    (gpsimd partition_broadcast + fused DVE affine_mul_reduce against
    the resident mvT16 tiles) and context^T output (PE transpose) are
    emitted inside batches b+1/b+2's A-loops, so the in-order ACT/DVE
    queues never make the PE wait at batch boundaries.
  - DMA rails: ACT's queue carries only the four tiny prologue loads
    (DMA trigger instructions cost ~0.7us of engine time and would
    stall the tanh stream); all bulk traffic rides the SP HWDGE rail
    (weights blocks interleaved in first-use order, then even batches)
    and the gpsimd SWDGE rail (odd batches).
  - tq runs once as fp8 DoubleRow matmuls interleaved into batch 0's
    A-phase, while the Ws8 blocks stream in.
  - a short PE warmup (pinned first via a PSUM WAW dep + explicit
    ordering edges) keeps the PE clock up while the prologue DMAs land.
"""

import sys

sys.path.insert(0, "/opt/trn_rl_repo")

from contextlib import ExitStack

import numpy as np

N_CORES = 8
B = 64
B_LOC = B // N_CORES  # 8 batches per core
N = 512
MD = 1024
QD = 1024
AD = 1024
P = 128
NG = 4         # DoubleRow K-groups over md/qd (4 x 256 = 1024)
NAD = AD // P  # 8 ad chunks
NMD = MD // P  # 8 md chunks
S_MV = 32.0
S_WH = 4096.0
S_Q = 32.0
S_WS = 4096.0
INV_A = 1.0 / (S_MV * S_WH)   # 2^-17, folded into tanh scale
INV_TQ = 1.0 / (S_Q * S_WS)   # 2^-17, folded into tq copy scale
WARMUP_MMS = 28
LAG = 3        # logits matmul lag (in A-groups) behind the tanh

_CACHE = {}


def _build_nc(npad):
    import concourse.bass as bass  # noqa: F401
    import concourse.tile as tile
    from concourse import bacc, mybir
    from concourse.masks import make_identity

    F32 = mybir.dt.float32
    F16 = mybir.dt.float16
    F8 = mybir.dt.float8e4
    AF = mybir.ActivationFunctionType
    OP = mybir.AluOpType
    AX = mybir.AxisListType
    DR = mybir.MatmulPerfMode.DoubleRow

    nc = bacc.Bacc("TRN2", target_bir_lowering=False)

    mvT8_d = nc.dram_tensor("mvT8", (B_LOC, P, NG, 2, npad), F8,
                            kind="ExternalInput")
    mvT16_d = nc.dram_tensor("mvT16", (B_LOC, P, NMD, npad), F16,
                             kind="ExternalInput")
    Wh8_d = nc.dram_tensor("Wh8", (P, NAD, NG, 2, P), F8,
                           kind="ExternalInput")
    Ws8_d = nc.dram_tensor("Ws8", (P, NAD, NG, 2, P), F8,
                           kind="ExternalInput")
    qT8_d = nc.dram_tensor("qT8", (P, NG, 2, B_LOC), F8,
                           kind="ExternalInput")
    v_d = nc.dram_tensor("v16", (P, NAD), F16, kind="ExternalInput")
    sup_d = nc.dram_tensor("sup", (1, B_LOC, npad), F32,
                           kind="ExternalInput")
    mx_d = nc.dram_tensor("mx", (1, B_LOC), F32, kind="ExternalInput")
    mvN_d = nc.dram_tensor("mvN", (2, npad, MD), F16, kind="ExternalInput")
    ctx_d = nc.dram_tensor("context", (B_LOC, MD), F32,
                           kind="ExternalOutput")
    scratch_d = nc.dram_tensor("scratch", (1, 2), F32,
                               kind="ExternalOutput")
    nch = -(-npad // P)  # row chunks of the natural-layout mv tail tiles

    with tile.TileContext(nc) as tc, ExitStack() as ctx:
        const = ctx.enter_context(tc.tile_pool(name="const", bufs=1))
        mvT8_pool = ctx.enter_context(tc.tile_pool(name="mvT8", bufs=3))
        mvT16_pool = ctx.enter_context(tc.tile_pool(name="mvT16", bufs=3))
        hT_pool = ctx.enter_context(tc.tile_pool(name="hT", bufs=6))
        small = ctx.enter_context(tc.tile_pool(name="small", bufs=2))
        dpool = ctx.enter_context(tc.tile_pool(name="dpool", bufs=3))
        out_pool = ctx.enter_context(tc.tile_pool(name="outp", bufs=2))
        natN_pool = ctx.enter_context(tc.tile_pool(name="natN", bufs=2))
        psum_h = ctx.enter_context(
            tc.tile_pool(name="psum_h", bufs=4, space="PSUM"))
        psum_log = ctx.enter_context(
            tc.tile_pool(name="psum_log", bufs=2, space="PSUM"))
        psum_tr = ctx.enter_context(
            tc.tile_pool(name="psum_tr", bufs=1, space="PSUM"))
        psum_keep = ctx.enter_context(
            tc.tile_pool(name="psum_keep", bufs=1, space="PSUM"))

        # ---- identities + PE warmup (keeps the PE clock up while the
        # ---- prologue DMAs stream in) -------------------------------------
        ident_f = const.tile([P, P], F32)
        make_identity(nc, ident_f[:])
        ident_h = const.tile([P, P], F16)
        make_identity(nc, ident_h[:])

        import bass_rust as _br

        ps_h0 = psum_h.tile([P, npad], F32, name="ps_h", tag="ps_h")
        last_warm = None
        for _ in range(WARMUP_MMS):
            last_warm = nc.tensor.matmul(ps_h0[:, 0:P], ident_h[:],
                                         ident_h[:], start=True, stop=True,
                                         skip_group_check=True)

        def after_warmup(bi):
            _br.add_dep_helper(bi.ins, last_warm.ins, sync=False,
                               reason="keep warmup at the head of the PE stream")
            return bi

        # ---- tiny loads: the ONLY traffic on the ACT rail ------------------
        qT8_sb = const.tile([P, NG, 2, B_LOC], F8)
        nc.scalar.dma_start(qT8_sb[:], qT8_d[:])
        v_sb = const.tile([P, NAD], F16)
        nc.scalar.dma_start(v_sb[:], v_d[:])
        sup_sb = const.tile([1, B_LOC, npad], F32)
        nc.scalar.dma_start(sup_sb[:], sup_d[:])
        mx_sb = const.tile([1, B_LOC], F32)
        nc.scalar.dma_start(mx_sb[:], mx_d[:])

        # ---- SP-rail prologue: weight blocks interleaved in first-use
        # ---- order, then batch-0 mv tiles ---------------------------------
        Ws8_sb = const.tile([P, NAD, NG, 2, P], F8)
        Wh8_sb = const.tile([P, NAD, NG, 2, P], F8)
        # Ws8 rides the ACT HWDGE rail: its trigger instructions execute
        # during the preamble while the ACT queue is otherwise idle, so
        # they never block the tanh stream
        nc.scalar.dma_start(Ws8_sb[:, 0:1], Ws8_d[:, 0:1])
        nc.scalar.dma_start(Ws8_sb[:, 1:2], Ws8_d[:, 1:2])
        nc.scalar.dma_start(Ws8_sb[:, 2:4], Ws8_d[:, 2:4])
        nc.scalar.dma_start(Ws8_sb[:, 4:8], Ws8_d[:, 4:8])
        nc.sync.dma_start(Wh8_sb[:, 0:2], Wh8_d[:, 0:2])

        mvT8s = [None] * B_LOC
        mvT16s = [None] * B_LOC

        def emit_loads(b):
            """mvT8 (A-phase) + mvT16 (context) loads for batch b.

            Batch 0 rides the SP HWDGE rail in g-chunks (its first chunk
            gates the first real matmul); odd batches ride the SWDGE
            rail, later even batches the SP rail, always with a full
            batch of slack."""
            mvT8 = mvT8_pool.tile([P, NG, 2, npad], F8, tag="mvT8")
            mvT16 = mvT16_pool.tile([P, NMD, npad], F16, tag="mvT16")
            if b == 0:
                for g in range(NG):
                    nc.sync.dma_start(mvT8[:, g], mvT8_d[b, :, g])
            elif b % 2 == 1:
                nc.gpsimd.dma_start(mvT8[:], mvT8_d[b])
                nc.gpsimd.dma_start(mvT16[:], mvT16_d[b])
            else:
                nc.sync.dma_start(mvT8[:], mvT8_d[b])
                nc.sync.dma_start(mvT16[:], mvT16_d[b])
            mvT8s[b] = mvT8
            mvT16s[b] = mvT16

        emit_loads(0)
        nc.sync.dma_start(Wh8_sb[:, 2:4], Wh8_d[:, 2:4])
        nc.sync.dma_start(Wh8_sb[:, 4:8], Wh8_d[:, 4:8])
        nc.sync.dma_start(mvT16s[0][:], mvT16_d[0])

        ones_h = const.tile([1, P], F16)
        nc.gpsimd.memset(ones_h[:], 1.0)

        # HAM keep-alive: the PE downclocks to half speed (k=4/8) within
        # ~4us of going idle, which would stretch every op in a stalled
        # region ~2x. Dummy matmuls into a scratch PSUM bank burn the
        # idle slots of DMA-gated (b0) and drain (tail) phases so the
        # clock stays up; they cost nothing when the PE queue is full.
        # They ACCUMULATE into one never-closed PSUM group that a final
        # read drains to a scratch output, so dead-code elimination
        # cannot drop them.
        ps_keep = psum_keep.tile([P, P], F32, name="ps_keep")
        ka_open = [False]

        def keep_alive(n, close=False):
            for k in range(n):
                nc.tensor.matmul(ps_keep[:], ident_h[:], ident_h[:],
                                 start=not ka_open[0],
                                 stop=close and k == n - 1,
                                 skip_group_check=True)
                ka_open[0] = True

        # ---- tq^T columns, fp8 DoubleRow, interleaved into batch 0 --------
        tqT_sb = const.tile([P, NAD, B_LOC], F32)

        def emit_tq(adc):
            ps_tq = psum_tr.tile([P, B_LOC], F32, tag="tr", name="ps_tq")
            for g in range(NG):
                after_warmup(nc.tensor.matmul(
                    ps_tq[:], Ws8_sb[:, adc, g], qT8_sb[:, g],
                    start=(g == 0), stop=(g == NG - 1), perf_mode=DR,
                    skip_group_check=True))
            nc.scalar.activation(tqT_sb[:, adc, :], ps_tq[:], AF.Copy,
                                 scale=INV_TQ)

        wbs = [None] * B_LOC
        ctxs = [None] * B_LOC
        ps_logs = [None] * B_LOC
        hts = {}

        def emit_logit(gi):
            bb, k = divmod(gi, NAD)
            nc.tensor.matmul(ps_logs[bb][:], v_sb[:, k:k + 1], hts.pop(gi),
                             start=(k == 0), stop=(k == NAD - 1),
                             skip_group_check=True)

        def emit_softmax(b):
            """masked softmax on partition 0 (sup/mx host-precomputed).

            No max-subtraction: |logits| <= ||v||_1 ~ 18 so fp32 exp
            cannot overflow, and suppressed (-1e30) entries underflow to
            exactly 0. The 1e-30 epsilon keeps 1/sum finite in the
            all-masked edge case (weights are then zeroed via mx)."""
            ml = small.tile([1, npad], F32, tag="ml")
            nc.vector.scalar_tensor_tensor(
                ml[:], in0=sup_sb[0:1, b, :], scalar=mx_sb[0:1, b:b + 1],
                in1=ps_logs[b][:], op0=OP.mult, op1=OP.add)
            et = small.tile([1, npad], F32, tag="et")
            zs = small.tile([1, 1], F32, tag="zs")
            nc.scalar.activation(et[:], ml[:], AF.Exp, accum_out=zs[:])
            zse = small.tile([1, 1], F32, tag="zse")
            nc.vector.tensor_scalar(zse[:], zs[:], 1.0, 1e-30,
                                    op0=OP.mult, op1=OP.add)
            rz = small.tile([1, 1], F32, tag="rz")
            nc.vector.reciprocal(rz[:], zse[:])
            wb = small.tile([1, npad], F16, tag="wb")
            nc.vector.tensor_scalar(wb[:], et[:], rz[:],
                                    mx_sb[0:1, b:b + 1],
                                    op0=OP.mult, op1=OP.mult)
            wbs[b] = wb

        def emit_D_compute(b):
            """context^T[md, b]: broadcast w(b) across partitions via a
            rank-1 PE matmul (ones x w) + ACT copy (both engines have
            slack), then one fused DVE multiply+reduce per md chunk
            against the resident mvT16."""
            ps_wbc = psum_tr.tile([P, npad], F32, tag="tr", name="ps_wbc")
            nc.tensor.matmul(ps_wbc[:], ones_h[:], wbs[b][:],
                             start=True, stop=True, skip_group_check=True)
            wbc = dpool.tile([P, npad], F16, tag="wbc")
            nc.scalar.copy(wbc[:], ps_wbc[:])
            ctx_b = dpool.tile([P, NMD], F32, tag="ctxb")
            for mdc in range(NMD):
                scr = dpool.tile([P, npad], F16, tag="dscr")
                nc.vector.affine_mul_reduce(scr[:], ctx_b[:, mdc:mdc + 1],
                                            mvT16s[b][:, mdc], wbc[:],
                                            1.0, 0.0)
            ctxs[b] = ctx_b

        def emit_D_out(b):
            """ctx^T [128(md_lo), 8(mdc)] -> [8, 128] -> DRAM."""
            ps_c = psum_tr.tile([B_LOC, P], F32, tag="tr", name="ps_c")
            nc.tensor.transpose(ps_c[:], ctxs[b][:], ident_f[:P, :P])
            out_sb = out_pool.tile([NMD, P], F32)
            nc.scalar.copy(out_sb[:], ps_c[:])
            nc.sync.dma_start(
                ctx_d[b:b + 1, :].rearrange("x (c p) -> (x c) p", p=P),
                out_sb[:])

        # ---- natural-layout mv for the last two batches: their context
        # ---- runs on the (tail-idle) PE instead of the DVE ----------------
        natNs = [None, None]

        def emit_natN(j):
            natN = natN_pool.tile([P, nch, MD], F16, tag="natN")
            for t in range(nch):
                r0, r1 = t * P, min((t + 1) * P, npad)
                nc.sync.dma_start(natN[0:r1 - r0, t, :],
                                  mvN_d[j, r0:r1, :])
            natNs[j] = natN

        def emit_D_pe(b, j):
            """context[b] = w @ mv_nat via M=1 PE matmuls (tail path)."""
            ps_wT = psum_tr.tile([P, nch, 2], F16, tag="tr", name="ps_wT")
            for t in range(nch):
                r0, r1 = t * P, min((t + 1) * P, npad)
                nc.tensor.matmul(ps_wT[0:r1 - r0, t, 0:1],
                                 wbs[b][0:1, r0:r1], ident_h[0:1, 0:1],
                                 is_transpose=True, skip_group_check=True)
            wT = dpool.tile([P, nch, 2], F16, tag="wT")
            nc.scalar.copy(wT[:, :, 0:1], ps_wT[:, :, 0:1])
            keep_alive(4)
            out_nat = out_pool.tile([1, MD], F32, name="out_nat",
                                    tag="onat")
            for half in range(2):
                ps_c2 = psum_tr.tile([1, MD // 2], F32, tag="tr",
                                     name="ps_c2")
                for t in range(nch):
                    r0, r1 = t * P, min((t + 1) * P, npad)
                    nc.tensor.matmul(
                        ps_c2[:], wT[0:r1 - r0, t, 0:1],
                        natNs[j][0:r1 - r0, t,
                                 half * (MD // 2):(half + 1) * (MD // 2)],
                        start=(t == 0), stop=(t == nch - 1),
                        skip_group_check=True)
                nc.scalar.copy(
                    out_nat[0:1, half * (MD // 2):(half + 1) * (MD // 2)],
                    ps_c2[:])
            nc.sync.dma_start(ctx_d[b:b + 1, :], out_nat[:])

        for b in range(B_LOC):
            if b + 1 < B_LOC:
                emit_loads(b + 1)

            mvT8 = mvT8s[b]
            ps_logs[b] = psum_log.tile([1, npad], F32, tag="log",
                                       name="ps_log")
            for adc in range(NAD):
                gi = b * NAD + adc
                if b == 0:
                    emit_tq(adc)
                if gi == 0:
                    ps_h = ps_h0
                else:
                    ps_h = psum_h.tile([P, npad], F32, name="ps_h",
                                       tag="ps_h")
                for g in range(NG):
                    mm = nc.tensor.matmul(
                        ps_h[:], Wh8_sb[:, adc, g], mvT8[:, g],
                        start=(g == 0), stop=(g == NG - 1), perf_mode=DR,
                        skip_group_check=True)
                    if b == 0:
                        after_warmup(mm)
                hT = hT_pool.tile([P, npad], F16)
                nc.scalar.activation(hT[:], ps_h[:], AF.Tanh,
                                     bias=tqT_sb[:, adc, b:b + 1],
                                     scale=INV_A)
                hts[gi] = hT
                if b == 0:
                    keep_alive(8)
                if gi >= LAG:
                    emit_logit(gi - LAG)
                if b > 0:
                    if adc == 3:
                        emit_softmax(b - 1)
                    elif adc == 5 and b - 1 < B_LOC - 2:
                        emit_D_compute(b - 1)
                    elif adc == 1 and b > 1 and b - 2 < B_LOC - 2:
                        emit_D_out(b - 2)
                    elif adc == 6 and b in (B_LOC - 3, B_LOC - 2):
                        emit_natN(b - (B_LOC - 3))

        for gi in range(B_LOC * NAD - LAG, B_LOC * NAD):
            emit_logit(gi)
        # tail: softmax(7) enters the DVE queue first; batch 6's context
        # runs on the now-idle PE in parallel (wb(6) ready since adc 3);
        # keep-alive bursts sit BEFORE each dependency stall in the
        # in-order PE queue so the clock stays up through the drain
        keep_alive(2)
        emit_softmax(B_LOC - 1)
        emit_D_pe(B_LOC - 2, 0)
        keep_alive(12)
        emit_D_pe(B_LOC - 1, 1)
        keep_alive(6, close=True)
        ka_sb = out_pool.tile([1, 2], F32, name="ka_sb")
        nc.vector.tensor_copy(ka_sb[:], ps_keep[0:1, 0:2])
        nc.sync.dma_start(scratch_d[:], ka_sb[:])

    nc.compile()
    return nc


def _get_nc(npad):
    key = ("nc", npad)
    if key not in _CACHE:
        _CACHE[key] = _build_nc(npad)
    return _CACHE[key]


def _quant8(x, scale):
    import ml_dtypes

    return np.clip(x * scale, -240.0, 240.0).astype(ml_dtypes.float8_e4m3)


def _prep(memory_values, mask, query, Wh, Ws, v):
    """Host-side dtype/layout prep. Returns (nc, per-core input maps)."""
    memory_values = np.ascontiguousarray(memory_values, dtype=np.float32)
    mask = np.ascontiguousarray(mask)
    query = np.ascontiguousarray(query, dtype=np.float32)
    Wh = np.ascontiguousarray(Wh, dtype=np.float32)
    Ws = np.ascontiguousarray(Ws, dtype=np.float32)
    v = np.ascontiguousarray(v, dtype=np.float32)

    # ---- host prep: gather unmasked rows to the front ----------------------
    unmasked = mask != 0
    counts = unmasked.sum(axis=1).astype(np.int64)
    npad = int(min(N, max(192, -(-int(counts.max()) // 32) * 32)))
    # unmasked rows first (stable order); pads are real masked rows whose
    # logits the suppression vector kills, so no zero-fill is needed.
    order = np.argsort(~unmasked, axis=1, kind="stable")[:, :npad]
    mv_g = np.take_along_axis(memory_values, order[:, :, None], axis=1)

    mv8 = _quant8(mv_g, S_MV)                       # [B, npad, MD]
    mvT8 = mv8.reshape(B, npad, NG, 2, P).transpose(0, 4, 2, 3, 1)
    mvT8 = np.ascontiguousarray(mvT8)               # [B, P, NG, 2, npad]
    mv16 = mv_g.astype(np.float16)
    mvT16 = mv16.reshape(B, npad, NMD, P).transpose(0, 3, 2, 1)
    mvT16 = np.ascontiguousarray(mvT16)             # [B, P, NMD, npad]

    # weights: [k, a] -> [p, adc, g, i, a_lo] with k = g*256 + i*128 + p
    def wprep(W, scale):
        w8 = _quant8(W, scale).reshape(NG, 2, P, NAD, P)
        return np.ascontiguousarray(w8.transpose(2, 3, 0, 1, 4))

    Wh8 = wprep(Wh, S_WH)
    Ws8 = wprep(Ws, S_WS)
    qT8 = np.ascontiguousarray(
        _quant8(query, S_Q).reshape(B, NG, 2, P).transpose(3, 1, 2, 0))
    v16 = np.ascontiguousarray(v[:, 0].reshape(NAD, P).T.astype(np.float16))

    sup = np.where(np.arange(npad)[None, :] < counts[:, None],
                   np.float32(0.0), np.float32(-1e30))
    mx = (counts > 0).astype(np.float32)

    nc = _get_nc(npad)
    in_maps = []
    for c in range(N_CORES):
        s = slice(c * B_LOC, (c + 1) * B_LOC)
        in_maps.append({
            "mvT8": mvT8[s],
            "mvT16": mvT16[s],
            "Wh8": Wh8,
            "Ws8": Ws8,
            "qT8": qT8[:, :, :, s],
            "v16": v16,
            "sup": sup[None, s],
            "mx": mx[None, s],
            "mvN": mv16[s][B_LOC - 2:],
        })
    return nc, in_maps


def kernel(memory_values, mask, query, Wh, Ws, v):
    from concourse.bass_utils import run_bass_kernel_spmd

    nc, in_maps = _prep(memory_values, mask, query, Wh, Ws, v)
    res = run_bass_kernel_spmd(nc, in_maps, core_ids=list(range(N_CORES)))
    out = np.concatenate([res.results[c]["context"] for c in range(N_CORES)],
                         axis=0)
    return out.astype(np.float32)


# revision 51
# speedup vs baseline: 1.2338x; 1.1811x over previous
"""Trainium2 Bass kernel for nn_Attention_50354196578449 (sparse_attention).

Reference computation (per batch b of B=64, N=512, MD=QD=AD=1024):
    tq      = query @ Ws                                   # (B, AD)
    h       = tanh(memory_values @ Wh + tq[:, None, :])    # (B, N, AD)
    logits  = squeeze(h @ v)                               # (B, N)
    weights = masked softmax(logits)                       # (B, N)
    context = einsum("bn,bnd->bd", weights, memory_values) # (B, MD)

Strategy: data-parallel over batch across 8 NeuronCores (8 batches/core).

Two levers on top of the fused fp16 pipeline:
  - sparsity: rows with mask==0 get -1e30 logits, so their h/logit work is
    dead. Host-side we gather each batch's unmasked rows to the front
    (padding with masked rows, suppressed via a host-built additive
    vector) and only compute NPAD ~ 288 of the 512 rows on device.
  - fp8 DoubleRow: the A-phase (mv @ Wh) and tq (q @ Ws) run as
    float8e4 matmuls in DoubleRow perf mode (2 K-tiles per instruction,
    0.5 cyc/row => 2x PE throughput). Inputs are pre-scaled by powers of
    two (mv,q x32; Wh,Ws x4096) so the fp8e4 normal range is used; the
    2^-17 product scale is folded into the tanh activation's scale and
    the tq PSUM->SBUF copy. Everything downstream of tanh (logits via
    fp16 h @ v, softmax in fp32, context via fp16 mv) stays >= fp16,
    which the softmax/context accuracy actually needs.

Per core, fully on-chip, software-pipelined across engines:
  - host pre-transposes everything (no on-chip transposes at all):
    mvT8 [p(md_lo), g, i, n] fp8 for the A-phase moving operand,
    mvT16 [p(md_lo), mdc, n] fp16 for the DVE context contraction,
    Wh8/Ws8 [p, adc, g, i, a_lo] fp8 stationary tiles, qT8, v16.
  - A-phase: per (batch, adc) one PSUM tile [128, NPAD] accumulates 4
    DoubleRow matmuls (K=256 each); ACT applies tanh with scale=2^-17
    and per-partition bias tq^T[:, b] -> hT fp16.
  - logits accumulate in PSUM via v-chunk (M=1) fp16 matmuls over ad,
    lagging the A-phase by three chunks so the ~600ns tanh and its
    semaphore hops never stall the in-order PE queue.
  - batch b's softmax (DVE/ACT/gpsimd small ops), context contraction
    (gpsimd partition_broadcast + fused DVE affine_mul_reduce against
    the resident mvT16 tiles) and context^T output (PE transpose) are
    emitted inside batches b+1/b+2's A-loops, so the in-order ACT/DVE
    queues never make the PE wait at batch boundaries.
  - DMA rails: ACT's queue carries only the four tiny prologue loads
    (DMA trigger instructions cost ~0.7us of engine time and would
    stall the tanh stream); all bulk traffic rides the SP HWDGE rail
    (weights blocks interleaved in first-use order, then even batches)
    and the gpsimd SWDGE rail (odd batches).
  - tq runs once as fp8 DoubleRow matmuls interleaved into batch 0's
    A-phase, while the Ws8 blocks stream in.
  - a short PE warmup (pinned first via a PSUM WAW dep + explicit
    ordering edges) keeps the PE clock up while the prologue DMAs land.
"""

import sys

sys.path.insert(0, "/opt/trn_rl_repo")

from contextlib import ExitStack

import numpy as np

N_CORES = 8
B = 64
B_LOC = B // N_CORES  # 8 batches per core
N = 512
MD = 1024
QD = 1024
AD = 1024
P = 128
NG = 4         # DoubleRow K-groups over md/qd (4 x 256 = 1024)
NAD = AD // P  # 8 ad chunks
NMD = MD // P  # 8 md chunks
S_MV = 32.0
S_WH = 4096.0
S_Q = 32.0
S_WS = 4096.0
S_V = 4096.0                  # v scale; folded into the exp activation
INV_A = 1.0 / (S_MV * S_WH)   # 2^-17, folded into tanh scale
INV_TQ = 1.0 / (S_Q * S_WS)   # 2^-17, folded into tq copy scale
SUPPRESS = -1e34              # pre-scaled -inf stand-in (exp scale 2^-12)
WARMUP_MMS = 28
LAG = 3        # tanh lag (in A-groups) behind which logit pairs trail
PLAG = 2       # logit DR-pair lag (in pairs = 2 A-groups each)

_CACHE = {}


def _build_nc(npad):
    import concourse.bass as bass  # noqa: F401
    import concourse.tile as tile
    from concourse import bacc, mybir
    from concourse.masks import make_identity

    F32 = mybir.dt.float32
    F16 = mybir.dt.float16
    F8 = mybir.dt.float8e4
    AF = mybir.ActivationFunctionType
    OP = mybir.AluOpType
    AX = mybir.AxisListType
    DR = mybir.MatmulPerfMode.DoubleRow

    nc = bacc.Bacc("TRN2", target_bir_lowering=False)

    mvT8_d = nc.dram_tensor("mvT8", (B_LOC, P, NG, 2, npad), F8,
                            kind="ExternalInput")
    mvT16_d = nc.dram_tensor("mvT16", (B_LOC, P, NMD, npad), F16,
                             kind="ExternalInput")
    Wh8_d = nc.dram_tensor("Wh8", (P, NAD, NG, 2, P), F8,
                           kind="ExternalInput")
    Ws8_d = nc.dram_tensor("Ws8", (P, NAD, NG, 2, P), F8,
                           kind="ExternalInput")
    qT8_d = nc.dram_tensor("qT8", (P, NG, 2, B_LOC), F8,
                           kind="ExternalInput")
    # DoubleRow LDWEIGHTS needs stationary M >= 16; pad v with zero
    # columns (the logits land in PSUM row 0, rows 1-15 are zeros)
    v_d = nc.dram_tensor("v8", (P, NAD // 2, 2, 16), F8,
                         kind="ExternalInput")
    sup_d = nc.dram_tensor("sup", (1, B_LOC, npad), F32,
                           kind="ExternalInput")
    mx_d = nc.dram_tensor("mx", (1, B_LOC), F32, kind="ExternalInput")
    mvN_d = nc.dram_tensor("mvN", (2, npad, MD), F16, kind="ExternalInput")
    ctx_d = nc.dram_tensor("context", (B_LOC, MD), F32,
                           kind="ExternalOutput")
    scratch_d = nc.dram_tensor("scratch", (1, 2), F32,
                               kind="ExternalOutput")
    nch = -(-npad // P)  # row chunks of the natural-layout mv tail tiles

    with tile.TileContext(nc) as tc, ExitStack() as ctx:
        const = ctx.enter_context(tc.tile_pool(name="const", bufs=1))
        mvT8_pool = ctx.enter_context(tc.tile_pool(name="mvT8", bufs=3))
        mvT16_pool = ctx.enter_context(tc.tile_pool(name="mvT16", bufs=3))
        hT_pool = ctx.enter_context(tc.tile_pool(name="hT", bufs=6))
        small = ctx.enter_context(tc.tile_pool(name="small", bufs=2))
        dpool = ctx.enter_context(tc.tile_pool(name="dpool", bufs=3))
        out_pool = ctx.enter_context(tc.tile_pool(name="outp", bufs=2))
        natN_pool = ctx.enter_context(tc.tile_pool(name="natN", bufs=2))
        psum_h = ctx.enter_context(
            tc.tile_pool(name="psum_h", bufs=4, space="PSUM"))
        psum_log = ctx.enter_context(
            tc.tile_pool(name="psum_log", bufs=2, space="PSUM"))
        psum_tr = ctx.enter_context(
            tc.tile_pool(name="psum_tr", bufs=1, space="PSUM"))
        psum_keep = ctx.enter_context(
            tc.tile_pool(name="psum_keep", bufs=1, space="PSUM"))

        # ---- identities + PE warmup (keeps the PE clock up while the
        # ---- prologue DMAs stream in) -------------------------------------
        ident_f = const.tile([P, P], F32)
        make_identity(nc, ident_f[:])
        ident_h = const.tile([P, P], F16)
        make_identity(nc, ident_h[:])

        import bass_rust as _br

        ps_h0 = psum_h.tile([P, npad], F32, name="ps_h", tag="ps_h")
        last_warm = None
        for _ in range(WARMUP_MMS):
            last_warm = nc.tensor.matmul(ps_h0[:, 0:P], ident_h[:],
                                         ident_h[:], start=True, stop=True,
                                         skip_group_check=True)

        def after_warmup(bi):
            _br.add_dep_helper(bi.ins, last_warm.ins, sync=False,
                               reason="keep warmup at the head of the PE stream")
            return bi

        # ---- tiny loads: the ONLY traffic on the ACT rail ------------------
        qT8_sb = const.tile([P, NG, 2, B_LOC], F8)
        nc.scalar.dma_start(qT8_sb[:], qT8_d[:])
        v_sb = const.tile([P, NAD // 2, 2, 16], F8)
        nc.scalar.dma_start(v_sb[:], v_d[:])
        sup_sb = const.tile([1, B_LOC, npad], F32)
        nc.scalar.dma_start(sup_sb[:], sup_d[:])
        mx_sb = const.tile([1, B_LOC], F32)
        nc.scalar.dma_start(mx_sb[:], mx_d[:])

        # ---- SP-rail prologue: weight blocks interleaved in first-use
        # ---- order, then batch-0 mv tiles ---------------------------------
        Ws8_sb = const.tile([P, NAD, NG, 2, P], F8)
        Wh8_sb = const.tile([P, NAD, NG, 2, P], F8)
        # Ws8 rides the ACT HWDGE rail: its trigger instructions execute
        # during the preamble while the ACT queue is otherwise idle, so
        # they never block the tanh stream
        nc.scalar.dma_start(Ws8_sb[:, 0:1], Ws8_d[:, 0:1])
        nc.scalar.dma_start(Ws8_sb[:, 1:2], Ws8_d[:, 1:2])
        nc.scalar.dma_start(Ws8_sb[:, 2:4], Ws8_d[:, 2:4])
        nc.scalar.dma_start(Ws8_sb[:, 4:8], Ws8_d[:, 4:8])
        nc.sync.dma_start(Wh8_sb[:, 0:2], Wh8_d[:, 0:2])

        mvT8s = [None] * B_LOC
        mvT16s = [None] * B_LOC

        def emit_loads(b):
            """mvT8 (A-phase) + mvT16 (context) loads for batch b.

            Batch 0 rides the SP HWDGE rail in g-chunks (its first chunk
            gates the first real matmul); odd batches ride the SWDGE
            rail, later even batches the SP rail, always with a full
            batch of slack."""
            mvT8 = mvT8_pool.tile([P, NG, 2, npad], F8, tag="mvT8")
            mvT16 = mvT16_pool.tile([P, NMD, npad], F16, tag="mvT16")
            if b == 0:
                for g in range(NG):
                    nc.sync.dma_start(mvT8[:, g], mvT8_d[b, :, g])
            elif b % 2 == 1:
                nc.gpsimd.dma_start(mvT8[:], mvT8_d[b])
                nc.gpsimd.dma_start(mvT16[:], mvT16_d[b])
            else:
                nc.sync.dma_start(mvT8[:], mvT8_d[b])
                nc.sync.dma_start(mvT16[:], mvT16_d[b])
            mvT8s[b] = mvT8
            mvT16s[b] = mvT16

        emit_loads(0)
        nc.sync.dma_start(Wh8_sb[:, 2:4], Wh8_d[:, 2:4])
        nc.sync.dma_start(Wh8_sb[:, 4:8], Wh8_d[:, 4:8])
        nc.sync.dma_start(mvT16s[0][:], mvT16_d[0])

        ones_h = const.tile([1, P], F16)
        nc.gpsimd.memset(ones_h[:], 1.0)

        # HAM keep-alive: the PE downclocks to half speed (k=4/8) within
        # ~4us of going idle, which would stretch every op in a stalled
        # region ~2x. Dummy matmuls into a scratch PSUM bank burn the
        # idle slots of DMA-gated (b0) and drain (tail) phases so the
        # clock stays up; they cost nothing when the PE queue is full.
        # They ACCUMULATE into one never-closed PSUM group that a final
        # read drains to a scratch output, so dead-code elimination
        # cannot drop them.
        ps_keep = psum_keep.tile([P, P], F32, name="ps_keep")
        ka_open = [False]

        def keep_alive(n, close=False):
            for k in range(n):
                nc.tensor.matmul(ps_keep[:], ident_h[:], ident_h[:],
                                 start=not ka_open[0],
                                 stop=close and k == n - 1,
                                 skip_group_check=True)
                ka_open[0] = True

        # ---- tq^T columns, fp8 DoubleRow, interleaved into batch 0 --------
        tqT_sb = const.tile([P, NAD, B_LOC], F32)

        def emit_tq(adc):
            ps_tq = psum_tr.tile([P, B_LOC], F32, tag="tr", name="ps_tq")
            for g in range(NG):
                after_warmup(nc.tensor.matmul(
                    ps_tq[:], Ws8_sb[:, adc, g], qT8_sb[:, g],
                    start=(g == 0), stop=(g == NG - 1), perf_mode=DR,
                    skip_group_check=True))
            nc.scalar.activation(tqT_sb[:, adc, :], ps_tq[:], AF.Copy,
                                 scale=INV_TQ)

        wbs = [None] * B_LOC
        ctxs = [None] * B_LOC
        ps_logs = [None] * B_LOC
        hts = {}

        def emit_logit(pig):
            """One fp8 DoubleRow logits matmul covering an adc PAIR."""
            bb, kp = divmod(pig, NAD // 2)
            nc.tensor.matmul(ps_logs[bb][:], v_sb[:, kp], hts.pop(pig),
                             start=(kp == 0), stop=(kp == NAD // 2 - 1),
                             perf_mode=DR, skip_group_check=True)

        def emit_softmax(b):
            """masked softmax on partition 0 (sup/mx host-precomputed).

            No max-subtraction: |logits| <= ||v||_1 ~ 18 so fp32 exp
            cannot overflow, and suppressed (-1e30) entries underflow to
            exactly 0. The 1e-30 epsilon keeps 1/sum finite in the
            all-masked edge case (weights are then zeroed via mx)."""
            ml = small.tile([1, npad], F32, tag="ml")
            nc.vector.scalar_tensor_tensor(
                ml[:], in0=sup_sb[0:1, b, :], scalar=mx_sb[0:1, b:b + 1],
                in1=ps_logs[b][0:1, :], op0=OP.mult, op1=OP.add)
            et = small.tile([1, npad], F32, tag="et")
            zs = small.tile([1, 1], F32, tag="zs")
            nc.scalar.activation(et[:], ml[:], AF.Exp, scale=1.0 / S_V,
                                 accum_out=zs[:])
            zse = small.tile([1, 1], F32, tag="zse")
            nc.vector.tensor_scalar(zse[:], zs[:], 1.0, 1e-30,
                                    op0=OP.mult, op1=OP.add)
            rz = small.tile([1, 1], F32, tag="rz")
            nc.vector.reciprocal(rz[:], zse[:])
            wb = small.tile([1, npad], F16, tag="wb")
            nc.vector.tensor_scalar(wb[:], et[:], rz[:],
                                    mx_sb[0:1, b:b + 1],
                                    op0=OP.mult, op1=OP.mult)
            wbs[b] = wb

        def emit_D_compute(b):
            """context^T[md, b]: broadcast w(b) across partitions via a
            rank-1 PE matmul (ones x w) + ACT copy (both engines have
            slack), then one fused DVE multiply+reduce per md chunk
            against the resident mvT16."""
            ps_wbc = psum_tr.tile([P, npad], F32, tag="tr", name="ps_wbc")
            nc.tensor.matmul(ps_wbc[:], ones_h[:], wbs[b][:],
                             start=True, stop=True, skip_group_check=True)
            wbc = dpool.tile([P, npad], F16, tag="wbc")
            nc.scalar.copy(wbc[:], ps_wbc[:])
            ctx_b = dpool.tile([P, NMD], F32, tag="ctxb")
            for mdc in range(NMD):
                scr = dpool.tile([P, npad], F16, tag="dscr")
                nc.vector.affine_mul_reduce(scr[:], ctx_b[:, mdc:mdc + 1],
                                            mvT16s[b][:, mdc], wbc[:],
                                            1.0, 0.0)
            ctxs[b] = ctx_b

        def emit_D_out(b):
            """ctx^T [128(md_lo), 8(mdc)] -> [8, 128] -> DRAM."""
            ps_c = psum_tr.tile([B_LOC, P], F32, tag="tr", name="ps_c")
            nc.tensor.transpose(ps_c[:], ctxs[b][:], ident_f[:P, :P])
            out_sb = out_pool.tile([NMD, P], F32)
            nc.scalar.copy(out_sb[:], ps_c[:])
            nc.sync.dma_start(
                ctx_d[b:b + 1, :].rearrange("x (c p) -> (x c) p", p=P),
                out_sb[:])

        # ---- natural-layout mv for the last two batches: their context
        # ---- runs on the (tail-idle) PE instead of the DVE ----------------
        natNs = [None, None]

        def emit_natN(j):
            natN = natN_pool.tile([P, nch, MD], F16, tag="natN")
            for t in range(nch):
                r0, r1 = t * P, min((t + 1) * P, npad)
                nc.sync.dma_start(natN[0:r1 - r0, t, :],
                                  mvN_d[j, r0:r1, :])
            natNs[j] = natN

        def emit_D_pe(b, j):
            """context[b] = w @ mv_nat via M=1 PE matmuls (tail path)."""
            ps_wT = psum_tr.tile([P, nch, 2], F16, tag="tr", name="ps_wT")
            for t in range(nch):
                r0, r1 = t * P, min((t + 1) * P, npad)
                nc.tensor.matmul(ps_wT[0:r1 - r0, t, 0:1],
                                 wbs[b][0:1, r0:r1], ident_h[0:1, 0:1],
                                 is_transpose=True, skip_group_check=True)
            wT = dpool.tile([P, nch, 2], F16, tag="wT")
            for t in range(nch):
                r0, r1 = t * P, min((t + 1) * P, npad)
                nc.scalar.copy(wT[0:r1 - r0, t, 0:1],
                               ps_wT[0:r1 - r0, t, 0:1])
            keep_alive(4)
            out_nat = out_pool.tile([1, MD], F32, name="out_nat",
                                    tag="onat")
            for half in range(2):
                ps_c2 = psum_tr.tile([1, MD // 2], F32, tag="tr",
                                     name="ps_c2")
                for t in range(nch):
                    r0, r1 = t * P, min((t + 1) * P, npad)
                    nc.tensor.matmul(
                        ps_c2[:], wT[0:r1 - r0, t, 0:1],
                        natNs[j][0:r1 - r0, t,
                                 half * (MD // 2):(half + 1) * (MD // 2)],
                        start=(t == 0), stop=(t == nch - 1),
                        skip_group_check=True)
                nc.scalar.copy(
                    out_nat[0:1, half * (MD // 2):(half + 1) * (MD // 2)],
                    ps_c2[:])
            nc.sync.dma_start(ctx_d[b:b + 1, :], out_nat[:])

        for b in range(B_LOC):
            if b + 1 < B_LOC:
                emit_loads(b + 1)

            mvT8 = mvT8s[b]
            ps_logs[b] = psum_log.tile([16, npad], F32, tag="log",
                                       name="ps_log")
            for adc in range(NAD):
                gi = b * NAD + adc
                if b == 0:
                    emit_tq(adc)
                if gi == 0:
                    ps_h = ps_h0
                else:
                    ps_h = psum_h.tile([P, npad], F32, name="ps_h",
                                       tag="ps_h")
                for g in range(NG):
                    mm = nc.tensor.matmul(
                        ps_h[:], Wh8_sb[:, adc, g], mvT8[:, g],
                        start=(g == 0), stop=(g == NG - 1), perf_mode=DR,
                        skip_group_check=True)
                    if b == 0:
                        after_warmup(mm)
                if adc % 2 == 0:
                    hT2 = hT_pool.tile([P, 2, npad], F8, name="hT2",
                                       tag="hT2")
                    hts[gi // 2] = hT2
                else:
                    hT2 = hts[gi // 2]
                nc.scalar.activation(hT2[:, adc % 2, :], ps_h[:], AF.Tanh,
                                     bias=tqT_sb[:, adc, b:b + 1],
                                     scale=INV_A)
                if b == 0:
                    keep_alive(8)
                if adc % 2 == 1 and gi // 2 >= PLAG:
                    emit_logit(gi // 2 - PLAG)
                if b > 0:
                    if adc == 4:
                        emit_softmax(b - 1)
                    elif adc == 6 and b - 1 < B_LOC - 2:
                        emit_D_compute(b - 1)
                    elif adc == 1 and b > 1 and b - 2 < B_LOC - 2:
                        emit_D_out(b - 2)
                    elif adc == 2 and b in (B_LOC - 3, B_LOC - 2):
                        emit_natN(b - (B_LOC - 3))

        for pig in range(B_LOC * (NAD // 2) - PLAG, B_LOC * (NAD // 2)):
            emit_logit(pig)
        # tail: softmax(7) enters the DVE queue first; batch 6's context
        # runs on the now-idle PE in parallel (wb(6) ready since adc 3);
        # keep-alive bursts sit BEFORE each dependency stall in the
        # in-order PE queue so the clock stays up through the drain
        keep_alive(2)
        emit_softmax(B_LOC - 1)
        emit_D_pe(B_LOC - 2, 0)
        keep_alive(12)
        emit_D_pe(B_LOC - 1, 1)
        keep_alive(6, close=True)
        ka_sb = out_pool.tile([1, 2], F32, name="ka_sb")
        nc.vector.tensor_copy(ka_sb[:], ps_keep[0:1, 0:2])
        nc.sync.dma_start(scratch_d[:], ka_sb[:])

    nc.compile()
    return nc


def _get_nc(npad):
    key = ("nc", npad)
    if key not in _CACHE:
        _CACHE[key] = _build_nc(npad)
    return _CACHE[key]


def _quant8(x, scale):
    import ml_dtypes

    return np.clip(x * scale, -240.0, 240.0).astype(ml_dtypes.float8_e4m3)


def _prep(memory_values, mask, query, Wh, Ws, v):
    """Host-side dtype/layout prep. Returns (nc, per-core input maps)."""
    memory_values = np.ascontiguousarray(memory_values, dtype=np.float32)
    mask = np.ascontiguousarray(mask)
    query = np.ascontiguousarray(query, dtype=np.float32)
    Wh = np.ascontiguousarray(Wh, dtype=np.float32)
    Ws = np.ascontiguousarray(Ws, dtype=np.float32)
    v = np.ascontiguousarray(v, dtype=np.float32)

    # ---- host prep: gather unmasked rows to the front ----------------------
    unmasked = mask != 0
    counts = unmasked.sum(axis=1).astype(np.int64)
    npad = int(min(N, max(192, -(-int(counts.max()) // 32) * 32)))
    # unmasked rows first (stable order); pads are real masked rows whose
    # logits the suppression vector kills, so no zero-fill is needed.
    order = np.argsort(~unmasked, axis=1, kind="stable")[:, :npad]
    mv_g = np.take_along_axis(memory_values, order[:, :, None], axis=1)

    mv8 = _quant8(mv_g, S_MV)                       # [B, npad, MD]
    mvT8 = mv8.reshape(B, npad, NG, 2, P).transpose(0, 4, 2, 3, 1)
    mvT8 = np.ascontiguousarray(mvT8)               # [B, P, NG, 2, npad]
    mv16 = mv_g.astype(np.float16)
    mvT16 = mv16.reshape(B, npad, NMD, P).transpose(0, 3, 2, 1)
    mvT16 = np.ascontiguousarray(mvT16)             # [B, P, NMD, npad]

    # weights: [k, a] -> [p, adc, g, i, a_lo] with k = g*256 + i*128 + p
    def wprep(W, scale):
        w8 = _quant8(W, scale).reshape(NG, 2, P, NAD, P)
        return np.ascontiguousarray(w8.transpose(2, 3, 0, 1, 4))

    Wh8 = wprep(Wh, S_WH)
    Ws8 = wprep(Ws, S_WS)
    qT8 = np.ascontiguousarray(
        _quant8(query, S_Q).reshape(B, NG, 2, P).transpose(3, 1, 2, 0))
    v8c = _quant8(v[:, 0], S_V).reshape(NAD // 2, 2, P).transpose(2, 0, 1)
    v8 = np.zeros((P, NAD // 2, 2, 16), dtype=v8c.dtype)
    v8[..., 0] = v8c

    sup = np.where(np.arange(npad)[None, :] < counts[:, None],
                   np.float32(0.0), np.float32(SUPPRESS))
    mx = (counts > 0).astype(np.float32)

    nc = _get_nc(npad)
    in_maps = []
    for c in range(N_CORES):
        s = slice(c * B_LOC, (c + 1) * B_LOC)
        in_maps.append({
            "mvT8": mvT8[s],
            "mvT16": mvT16[s],
            "Wh8": Wh8,
            "Ws8": Ws8,
            "qT8": qT8[:, :, :, s],
            "v8": v8,
            "sup": sup[None, s],
            "mx": mx[None, s],
            "mvN": mv16[s][B_LOC - 2:],
        })
    return nc, in_maps


def kernel(memory_values, mask, query, Wh, Ws, v):
    from concourse.bass_utils import run_bass_kernel_spmd

    nc, in_maps = _prep(memory_values, mask, query, Wh, Ws, v)
    res = run_bass_kernel_spmd(nc, in_maps, core_ids=list(range(N_CORES)))
    out = np.concatenate([res.results[c]["context"] for c in range(N_CORES)],
                         axis=0)
    return out.astype(np.float32)


# revision 53
# speedup vs baseline: 1.2738x; 1.0324x over previous
"""Trainium2 Bass kernel for nn_Attention_50354196578449 (sparse_attention).

Reference computation (per batch b of B=64, N=512, MD=QD=AD=1024):
    tq      = query @ Ws                                   # (B, AD)
    h       = tanh(memory_values @ Wh + tq[:, None, :])    # (B, N, AD)
    logits  = squeeze(h @ v)                               # (B, N)
    weights = masked softmax(logits)                       # (B, N)
    context = einsum("bn,bnd->bd", weights, memory_values) # (B, MD)

Strategy: data-parallel over batch across 8 NeuronCores (8 batches/core).

Two levers on top of the fused fp16 pipeline:
  - sparsity: rows with mask==0 get -1e30 logits, so their h/logit work is
    dead. Host-side we gather each batch's unmasked rows to the front
    (padding with masked rows, suppressed via a host-built additive
    vector) and only compute NPAD ~ 288 of the 512 rows on device.
  - fp8 DoubleRow: the A-phase (mv @ Wh) and tq (q @ Ws) run as
    float8e4 matmuls in DoubleRow perf mode (2 K-tiles per instruction,
    0.5 cyc/row => 2x PE throughput). Inputs are pre-scaled by powers of
    two (mv,q x32; Wh,Ws x4096) so the fp8e4 normal range is used; the
    2^-17 product scale is folded into the tanh activation's scale and
    the tq PSUM->SBUF copy. Everything downstream of tanh (logits via
    fp16 h @ v, softmax in fp32, context via fp16 mv) stays >= fp16,
    which the softmax/context accuracy actually needs.

Per core, fully on-chip, software-pipelined across engines:
  - host pre-transposes everything (no on-chip transposes at all):
    mvT8 [p(md_lo), g, i, n] fp8 for the A-phase moving operand,
    mvT16 [p(md_lo), mdc, n] fp16 for the DVE context contraction,
    Wh8/Ws8 [p, adc, g, i, a_lo] fp8 stationary tiles, qT8, v16.
  - A-phase: per (batch, adc) one PSUM tile [128, NPAD] accumulates 4
    DoubleRow matmuls (K=256 each); ACT applies tanh with scale=2^-17
    and per-partition bias tq^T[:, b] -> hT fp16.
  - logits accumulate in PSUM via v-chunk (M=1) fp16 matmuls over ad,
    lagging the A-phase by three chunks so the ~600ns tanh and its
    semaphore hops never stall the in-order PE queue.
  - batch b's softmax (DVE/ACT/gpsimd small ops), context contraction
    (gpsimd partition_broadcast + fused DVE affine_mul_reduce against
    the resident mvT16 tiles) and context^T output (PE transpose) are
    emitted inside batches b+1/b+2's A-loops, so the in-order ACT/DVE
    queues never make the PE wait at batch boundaries.
  - DMA rails: ACT's queue carries only the four tiny prologue loads
    (DMA trigger instructions cost ~0.7us of engine time and would
    stall the tanh stream); all bulk traffic rides the SP HWDGE rail
    (weights blocks interleaved in first-use order, then even batches)
    and the gpsimd SWDGE rail (odd batches).
  - tq runs once as fp8 DoubleRow matmuls interleaved into batch 0's
    A-phase, while the Ws8 blocks stream in.
  - a short PE warmup (pinned first via a PSUM WAW dep + explicit
    ordering edges) keeps the PE clock up while the prologue DMAs land.
"""

import sys

sys.path.insert(0, "/opt/trn_rl_repo")

from contextlib import ExitStack

import numpy as np

N_CORES = 8
B = 64
B_LOC = B // N_CORES  # 8 batches per core
N = 512
MD = 1024
QD = 1024
AD = 1024
P = 128
NG = 4         # DoubleRow K-groups over md/qd (4 x 256 = 1024)
NAD = AD // P  # 8 ad chunks
NMD = MD // P  # 8 md chunks
S_MV = 32.0
S_WH = 4096.0
S_Q = 32.0
S_WS = 4096.0
S_V = 4096.0                  # v scale; folded into the exp activation
INV_A = 1.0 / (S_MV * S_WH)   # 2^-17, folded into tanh scale
INV_TQ = 1.0 / (S_Q * S_WS)   # 2^-17, folded into tq copy scale
SUPPRESS = -1e34              # pre-scaled -inf stand-in (exp scale 2^-12)
WARMUP_MMS = 28
LAG = 3        # tanh lag (in A-groups) behind which logit pairs trail
PLAG = 2       # logit DR-pair lag (in pairs = 2 A-groups each)

_CACHE = {}


def _build_nc(npad):
    import concourse.bass as bass  # noqa: F401
    import concourse.tile as tile
    from concourse import bacc, mybir
    from concourse.masks import make_identity

    F32 = mybir.dt.float32
    F16 = mybir.dt.float16
    F8 = mybir.dt.float8e4
    AF = mybir.ActivationFunctionType
    OP = mybir.AluOpType
    AX = mybir.AxisListType
    DR = mybir.MatmulPerfMode.DoubleRow

    nc = bacc.Bacc("TRN2", target_bir_lowering=False)

    mvT8_d = nc.dram_tensor("mvT8", (B_LOC, P, NG, 2, npad), F8,
                            kind="ExternalInput")
    mvT16_d = nc.dram_tensor("mvT16", (B_LOC, P, NMD, npad), F16,
                             kind="ExternalInput")
    Wh8_d = nc.dram_tensor("Wh8", (P, NAD, NG, 2, P), F8,
                           kind="ExternalInput")
    Ws8_d = nc.dram_tensor("Ws8", (P, NAD, NG, 2, P), F8,
                           kind="ExternalInput")
    qT8_d = nc.dram_tensor("qT8", (P, NG, 2, B_LOC), F8,
                           kind="ExternalInput")
    # DoubleRow LDWEIGHTS needs stationary M >= 16; pad v with zero
    # columns (the logits land in PSUM row 0, rows 1-15 are zeros)
    v_d = nc.dram_tensor("v8", (P, NAD // 2, 2, 16), F8,
                         kind="ExternalInput")
    sup_d = nc.dram_tensor("sup", (1, B_LOC, npad), F32,
                           kind="ExternalInput")
    mx_d = nc.dram_tensor("mx", (1, B_LOC), F32, kind="ExternalInput")
    mvN_d = nc.dram_tensor("mvN", (2, npad, MD), F16, kind="ExternalInput")
    ctx_d = nc.dram_tensor("context", (B_LOC, MD), F32,
                           kind="ExternalOutput")
    scratch_d = nc.dram_tensor("scratch", (1, 2), F32,
                               kind="ExternalOutput")
    nch = -(-npad // P)  # row chunks of the natural-layout mv tail tiles

    with tile.TileContext(nc) as tc, ExitStack() as ctx:
        const = ctx.enter_context(tc.tile_pool(name="const", bufs=1))
        mvT8_pool = ctx.enter_context(tc.tile_pool(name="mvT8", bufs=3))
        mvT16_pool = ctx.enter_context(tc.tile_pool(name="mvT16", bufs=3))
        hT_pool = ctx.enter_context(tc.tile_pool(name="hT", bufs=6))
        small = ctx.enter_context(tc.tile_pool(name="small", bufs=2))
        dpool = ctx.enter_context(tc.tile_pool(name="dpool", bufs=3))
        out_pool = ctx.enter_context(tc.tile_pool(name="outp", bufs=2))
        natN_pool = ctx.enter_context(tc.tile_pool(name="natN", bufs=2))
        psum_h = ctx.enter_context(
            tc.tile_pool(name="psum_h", bufs=4, space="PSUM"))
        psum_log = ctx.enter_context(
            tc.tile_pool(name="psum_log", bufs=2, space="PSUM"))
        psum_tr = ctx.enter_context(
            tc.tile_pool(name="psum_tr", bufs=1, space="PSUM"))
        psum_keep = ctx.enter_context(
            tc.tile_pool(name="psum_keep", bufs=1, space="PSUM"))

        # ---- identities + PE warmup (keeps the PE clock up while the
        # ---- prologue DMAs stream in) -------------------------------------
        ident_f = const.tile([P, P], F32)
        make_identity(nc, ident_f[:])
        ident_h = const.tile([P, P], F16)
        make_identity(nc, ident_h[:])

        import bass_rust as _br

        ps_h0 = psum_h.tile([P, npad], F32, name="ps_h", tag="ps_h")
        last_warm = None
        for _ in range(WARMUP_MMS):
            last_warm = nc.tensor.matmul(ps_h0[:, 0:P], ident_h[:],
                                         ident_h[:], start=True, stop=True,
                                         skip_group_check=True)

        def after_warmup(bi):
            _br.add_dep_helper(bi.ins, last_warm.ins, sync=False,
                               reason="keep warmup at the head of the PE stream")
            return bi

        # ---- tiny loads: the ONLY traffic on the ACT rail ------------------
        qT8_sb = const.tile([P, NG, 2, B_LOC], F8)
        nc.scalar.dma_start(qT8_sb[:], qT8_d[:])
        v_sb = const.tile([P, NAD // 2, 2, 16], F8)
        nc.scalar.dma_start(v_sb[:], v_d[:])
        sup_sb = const.tile([1, B_LOC, npad], F32)
        nc.scalar.dma_start(sup_sb[:], sup_d[:])
        mx_sb = const.tile([1, B_LOC], F32)
        nc.scalar.dma_start(mx_sb[:], mx_d[:])

        # ---- SP-rail prologue: weight blocks interleaved in first-use
        # ---- order, then batch-0 mv tiles ---------------------------------
        Ws8_sb = const.tile([P, NAD, NG, 2, P], F8)
        Wh8_sb = const.tile([P, NAD, NG, 2, P], F8)
        # Ws8 rides the ACT HWDGE rail: its trigger instructions execute
        # during the preamble while the ACT queue is otherwise idle, so
        # they never block the tanh stream
        nc.scalar.dma_start(Ws8_sb[:, 0:1], Ws8_d[:, 0:1])
        nc.scalar.dma_start(Ws8_sb[:, 1:2], Ws8_d[:, 1:2])
        nc.scalar.dma_start(Ws8_sb[:, 2:4], Ws8_d[:, 2:4])
        nc.scalar.dma_start(Ws8_sb[:, 4:8], Ws8_d[:, 4:8])
        nc.sync.dma_start(Wh8_sb[:, 0:2], Wh8_d[:, 0:2])

        mvT8s = [None] * B_LOC
        mvT16s = [None] * B_LOC

        def emit_loads(b):
            """mvT8 (A-phase) + mvT16 (context) loads for batch b.

            Batch 0 rides the SP HWDGE rail in g-chunks (its first chunk
            gates the first real matmul); odd batches ride the SWDGE
            rail, later even batches the SP rail, always with a full
            batch of slack."""
            mvT8 = mvT8_pool.tile([P, NG, 2, npad], F8, tag="mvT8")
            mvT16 = mvT16_pool.tile([P, NMD, npad], F16, tag="mvT16")
            if b == 0:
                for g in range(NG):
                    nc.sync.dma_start(mvT8[:, g], mvT8_d[b, :, g])
            elif b % 2 == 1:
                nc.gpsimd.dma_start(mvT8[:], mvT8_d[b])
                nc.gpsimd.dma_start(mvT16[:], mvT16_d[b])
            else:
                nc.sync.dma_start(mvT8[:], mvT8_d[b])
                nc.sync.dma_start(mvT16[:], mvT16_d[b])
            mvT8s[b] = mvT8
            mvT16s[b] = mvT16

        emit_loads(0)
        nc.sync.dma_start(Wh8_sb[:, 2:4], Wh8_d[:, 2:4])
        nc.sync.dma_start(Wh8_sb[:, 4:8], Wh8_d[:, 4:8])
        nc.sync.dma_start(mvT16s[0][:], mvT16_d[0])

        ones_h = const.tile([1, P], F16)
        nc.gpsimd.memset(ones_h[:], 1.0)

        # HAM keep-alive: the PE downclocks to half speed (k=4/8) within
        # ~4us of going idle, which would stretch every op in a stalled
        # region ~2x. Dummy matmuls into a scratch PSUM bank burn the
        # idle slots of DMA-gated (b0) and drain (tail) phases so the
        # clock stays up; they cost nothing when the PE queue is full.
        # They ACCUMULATE into one never-closed PSUM group that a final
        # read drains to a scratch output, so dead-code elimination
        # cannot drop them.
        ps_keep = psum_keep.tile([P, P], F32, name="ps_keep")
        ka_open = [False]

        def keep_alive(n, close=False, after=None):
            for k in range(n):
                mm = nc.tensor.matmul(ps_keep[:], ident_h[:], ident_h[:],
                                      start=not ka_open[0],
                                      stop=close and k == n - 1,
                                      skip_group_check=True)
                ka_open[0] = True
                if after is not None and k == 0:
                    # ordering-only edge: stops the scheduler hoisting
                    # the (otherwise dependency-free) burst away from
                    # the idle window it is meant to fill
                    _br.add_dep_helper(mm.ins, after.ins, sync=False,
                                       reason="pin keep-alive burst")

        # ---- tq^T columns, fp8 DoubleRow, interleaved into batch 0 --------
        tqT_sb = const.tile([P, NAD, B_LOC], F32)

        def emit_tq(adc):
            ps_tq = psum_tr.tile([P, B_LOC], F32, tag="tr", name="ps_tq")
            for g in range(NG):
                after_warmup(nc.tensor.matmul(
                    ps_tq[:], Ws8_sb[:, adc, g], qT8_sb[:, g],
                    start=(g == 0), stop=(g == NG - 1), perf_mode=DR,
                    skip_group_check=True))
            nc.scalar.activation(tqT_sb[:, adc, :], ps_tq[:], AF.Copy,
                                 scale=INV_TQ)

        wbs = [None] * B_LOC
        ctxs = [None] * B_LOC
        ps_logs = [None] * B_LOC
        hts = {}

        def emit_logit(pig):
            """One fp8 DoubleRow logits matmul covering an adc PAIR."""
            bb, kp = divmod(pig, NAD // 2)
            return nc.tensor.matmul(
                ps_logs[bb][:], v_sb[:, kp], hts.pop(pig),
                start=(kp == 0), stop=(kp == NAD // 2 - 1),
                perf_mode=DR, skip_group_check=True)

        def emit_softmax(b):
            """masked softmax on partition 0 (sup/mx host-precomputed).

            No max-subtraction: |logits| <= ||v||_1 ~ 18 so fp32 exp
            cannot overflow, and suppressed (-1e30) entries underflow to
            exactly 0. The 1e-30 epsilon keeps 1/sum finite in the
            all-masked edge case (weights are then zeroed via mx)."""
            ml = small.tile([1, npad], F32, tag="ml")
            nc.vector.scalar_tensor_tensor(
                ml[:], in0=sup_sb[0:1, b, :], scalar=mx_sb[0:1, b:b + 1],
                in1=ps_logs[b][0:1, :], op0=OP.mult, op1=OP.add)
            et = small.tile([1, npad], F32, tag="et")
            zs = small.tile([1, 1], F32, tag="zs")
            nc.scalar.activation(et[:], ml[:], AF.Exp, scale=1.0 / S_V,
                                 accum_out=zs[:])
            zse = small.tile([1, 1], F32, tag="zse")
            nc.vector.tensor_scalar(zse[:], zs[:], 1.0, 1e-30,
                                    op0=OP.mult, op1=OP.add)
            rz = small.tile([1, 1], F32, tag="rz")
            nc.vector.reciprocal(rz[:], zse[:])
            wb = small.tile([1, npad], F16, tag="wb")
            nc.vector.tensor_scalar(wb[:], et[:], rz[:],
                                    mx_sb[0:1, b:b + 1],
                                    op0=OP.mult, op1=OP.mult)
            wbs[b] = wb

        def emit_D_compute(b):
            """context^T[md, b]: broadcast w(b) across partitions via a
            rank-1 PE matmul (ones x w) + ACT copy (both engines have
            slack), then one fused DVE multiply+reduce per md chunk
            against the resident mvT16."""
            ps_wbc = psum_tr.tile([P, npad], F32, tag="tr", name="ps_wbc")
            nc.tensor.matmul(ps_wbc[:], ones_h[:], wbs[b][:],
                             start=True, stop=True, skip_group_check=True)
            wbc = dpool.tile([P, npad], F16, tag="wbc")
            nc.scalar.copy(wbc[:], ps_wbc[:])
            ctx_b = dpool.tile([P, NMD], F32, tag="ctxb")
            for mdc in range(NMD):
                scr = dpool.tile([P, npad], F16, tag="dscr")
                nc.vector.affine_mul_reduce(scr[:], ctx_b[:, mdc:mdc + 1],
                                            mvT16s[b][:, mdc], wbc[:],
                                            1.0, 0.0)
            ctxs[b] = ctx_b

        def emit_D_out(b):
            """ctx^T [128(md_lo), 8(mdc)] -> [8, 128] -> DRAM."""
            ps_c = psum_tr.tile([B_LOC, P], F32, tag="tr", name="ps_c")
            nc.tensor.transpose(ps_c[:], ctxs[b][:], ident_f[:P, :P])
            out_sb = out_pool.tile([NMD, P], F32)
            nc.scalar.copy(out_sb[:], ps_c[:])
            nc.sync.dma_start(
                ctx_d[b:b + 1, :].rearrange("x (c p) -> (x c) p", p=P),
                out_sb[:])

        # ---- natural-layout mv for the last two batches: their context
        # ---- runs on the (tail-idle) PE instead of the DVE ----------------
        natNs = [None, None]

        def emit_natN(j):
            natN = natN_pool.tile([P, nch, MD], F16, tag="natN")
            for t in range(nch):
                r0, r1 = t * P, min((t + 1) * P, npad)
                nc.sync.dma_start(natN[0:r1 - r0, t, :],
                                  mvN_d[j, r0:r1, :])
            natNs[j] = natN

        def emit_D_pe(b, j):
            """context[b] = w @ mv_nat via M=1 PE matmuls (tail path)."""
            ps_wT = psum_tr.tile([P, nch, 2], F16, tag="tr", name="ps_wT")
            for t in range(nch):
                r0, r1 = t * P, min((t + 1) * P, npad)
                trm = nc.tensor.matmul(ps_wT[0:r1 - r0, t, 0:1],
                                       wbs[b][0:1, r0:r1],
                                       ident_h[0:1, 0:1],
                                       is_transpose=True,
                                       skip_group_check=True)
            wT = dpool.tile([P, nch, 2], F16, tag="wT")
            for t in range(nch):
                r0, r1 = t * P, min((t + 1) * P, npad)
                nc.scalar.copy(wT[0:r1 - r0, t, 0:1],
                               ps_wT[0:r1 - r0, t, 0:1])
            keep_alive(6, after=trm)
            out_nat = out_pool.tile([1, MD], F32, name="out_nat",
                                    tag="onat")
            for half in range(2):
                ps_c2 = psum_tr.tile([1, MD // 2], F32, tag="tr",
                                     name="ps_c2")
                for t in range(nch):
                    r0, r1 = t * P, min((t + 1) * P, npad)
                    cmm = nc.tensor.matmul(
                        ps_c2[:], wT[0:r1 - r0, t, 0:1],
                        natNs[j][0:r1 - r0, t,
                                 half * (MD // 2):(half + 1) * (MD // 2)],
                        start=(t == 0), stop=(t == nch - 1),
                        skip_group_check=True)
                nc.vector.tensor_copy(
                    out_nat[0:1, half * (MD // 2):(half + 1) * (MD // 2)],
                    ps_c2[:])
            nc.sync.dma_start(ctx_d[b:b + 1, :], out_nat[:])
            return cmm

        for b in range(B_LOC):
            if b + 1 < B_LOC:
                emit_loads(b + 1)

            mvT8 = mvT8s[b]
            ps_logs[b] = psum_log.tile([16, npad], F32, tag="log",
                                       name="ps_log")
            for adc in range(NAD):
                gi = b * NAD + adc
                if b == 0:
                    emit_tq(adc)
                if gi == 0:
                    ps_h = ps_h0
                else:
                    ps_h = psum_h.tile([P, npad], F32, name="ps_h",
                                       tag="ps_h")
                for g in range(NG):
                    mm = nc.tensor.matmul(
                        ps_h[:], Wh8_sb[:, adc, g], mvT8[:, g],
                        start=(g == 0), stop=(g == NG - 1), perf_mode=DR,
                        skip_group_check=True)
                    if b == 0:
                        after_warmup(mm)
                astop = mm
                if adc % 2 == 0:
                    hT2 = hT_pool.tile([P, 2, npad], F8, name="hT2",
                                       tag="hT2")
                    hts[gi // 2] = hT2
                else:
                    hT2 = hts[gi // 2]
                nc.scalar.activation(hT2[:, adc % 2, :], ps_h[:], AF.Tanh,
                                     bias=tqT_sb[:, adc, b:b + 1],
                                     scale=INV_A)
                if b == 0:
                    keep_alive(8, after=astop)
                if adc % 2 == 1 and gi // 2 >= PLAG:
                    emit_logit(gi // 2 - PLAG)
                if b > 0:
                    if adc == 4:
                        emit_softmax(b - 1)
                    elif adc == 6 and b - 1 < B_LOC - 2:
                        emit_D_compute(b - 1)
                    elif adc == 1 and b > 1 and b - 2 < B_LOC - 2:
                        emit_D_out(b - 2)
                    elif adc == 2 and b in (B_LOC - 3, B_LOC - 2):
                        emit_natN(b - (B_LOC - 3))

        for pig in range(B_LOC * (NAD // 2) - PLAG, B_LOC * (NAD // 2)):
            last_logit = emit_logit(pig)
        # tail: softmax(7) enters the DVE queue first; batch 6's context
        # runs on the now-idle PE in parallel (wb(6) ready since adc 3);
        # keep-alive bursts, pinned behind the preceding real PE work,
        # fill each dependency stall so the clock stays up to the end
        keep_alive(4, after=last_logit)
        emit_softmax(B_LOC - 1)
        d6 = emit_D_pe(B_LOC - 2, 0)
        keep_alive(14, after=d6)
        d7 = emit_D_pe(B_LOC - 1, 1)
        keep_alive(8, close=True, after=d7)
        ka_sb = out_pool.tile([1, 2], F32, name="ka_sb")
        nc.vector.tensor_copy(ka_sb[:], ps_keep[0:1, 0:2])
        nc.sync.dma_start(scratch_d[:], ka_sb[:])

    nc.compile()
    return nc


def _get_nc(npad):
    key = ("nc", npad)
    if key not in _CACHE:
        _CACHE[key] = _build_nc(npad)
    return _CACHE[key]


def _quant8(x, scale):
    import ml_dtypes

    return np.clip(x * scale, -240.0, 240.0).astype(ml_dtypes.float8_e4m3)


def _prep(memory_values, mask, query, Wh, Ws, v):
    """Host-side dtype/layout prep. Returns (nc, per-core input maps)."""
    memory_values = np.ascontiguousarray(memory_values, dtype=np.float32)
    mask = np.ascontiguousarray(mask)
    query = np.ascontiguousarray(query, dtype=np.float32)
    Wh = np.ascontiguousarray(Wh, dtype=np.float32)
    Ws = np.ascontiguousarray(Ws, dtype=np.float32)
    v = np.ascontiguousarray(v, dtype=np.float32)

    # ---- host prep: gather unmasked rows to the front ----------------------
    unmasked = mask != 0
    counts = unmasked.sum(axis=1).astype(np.int64)
    npad = int(min(N, max(192, -(-int(counts.max()) // 32) * 32)))
    # unmasked rows first (stable order); pads are real masked rows whose
    # logits the suppression vector kills, so no zero-fill is needed.
    order = np.argsort(~unmasked, axis=1, kind="stable")[:, :npad]
    mv_g = np.take_along_axis(memory_values, order[:, :, None], axis=1)

    mv8 = _quant8(mv_g, S_MV)                       # [B, npad, MD]
    mvT8 = mv8.reshape(B, npad, NG, 2, P).transpose(0, 4, 2, 3, 1)
    mvT8 = np.ascontiguousarray(mvT8)               # [B, P, NG, 2, npad]
    mv16 = mv_g.astype(np.float16)
    mvT16 = mv16.reshape(B, npad, NMD, P).transpose(0, 3, 2, 1)
    mvT16 = np.ascontiguousarray(mvT16)             # [B, P, NMD, npad]

    # weights: [k, a] -> [p, adc, g, i, a_lo] with k = g*256 + i*128 + p
    def wprep(W, scale):
        w8 = _quant8(W, scale).reshape(NG, 2, P, NAD, P)
        return np.ascontiguousarray(w8.transpose(2, 3, 0, 1, 4))

    Wh8 = wprep(Wh, S_WH)
    Ws8 = wprep(Ws, S_WS)
    qT8 = np.ascontiguousarray(
        _quant8(query, S_Q).reshape(B, NG, 2, P).transpose(3, 1, 2, 0))
    v8c = _quant8(v[:, 0], S_V).reshape(NAD // 2, 2, P).transpose(2, 0, 1)
    v8 = np.zeros((P, NAD // 2, 2, 16), dtype=v8c.dtype)
    v8[..., 0] = v8c

    sup = np.where(np.arange(npad)[None, :] < counts[:, None],
                   np.float32(0.0), np.float32(SUPPRESS))
    mx = (counts > 0).astype(np.float32)

    nc = _get_nc(npad)
    in_maps = []
    for c in range(N_CORES):
        s = slice(c * B_LOC, (c + 1) * B_LOC)
        in_maps.append({
            "mvT8": mvT8[s],
            "mvT16": mvT16[s],
            "Wh8": Wh8,
            "Ws8": Ws8,
            "qT8": qT8[:, :, :, s],
            "v8": v8,
            "sup": sup[None, s],
            "mx": mx[None, s],
            "mvN": mv16[s][B_LOC - 2:],
        })
    return nc, in_maps


def kernel(memory_values, mask, query, Wh, Ws, v):
    from concourse.bass_utils import run_bass_kernel_spmd

    nc, in_maps = _prep(memory_values, mask, query, Wh, Ws, v)
    res = run_bass_kernel_spmd(nc, in_maps, core_ids=list(range(N_CORES)))
    out = np.concatenate([res.results[c]["context"] for c in range(N_CORES)],
                         axis=0)
    return out.astype(np.float32)
